# revision 64
# baseline (speedup 1.0000x reference)
"""Trainium2 Bass kernel for nn_AbstractAttention (B=2, S=2048, D=1024, H=16, dh=64).

Sharding: 8 cores = 2 batch groups x 4 cores; core i handles batch i//4 and
heads 4*(i%4)..+4 (QKV projection + causal attention for those heads).  The
output projection is head-sharded: each core projects its own 4 heads' z into
a partial [512, 1024] output per query chunk (K=256), and a per-chunk 4-core
fp16 ReduceScatter sums the partials, leaving core j of each group exactly
query rows j*128 of the chunk.  The host reassembles the interleaved
row-slices from all 8 cores.

Per-core structure (pipelined per 512-position chunk):
  - host pre-transposes x to [d_model, seq] fp16 and pre-packs W_Q/K/V
    d-major so all input DMAs are contiguous (no DMA-transpose).
  - chunk pc: project K/V/Q for positions [512*pc, 512*pc+512) (fp16 PE,
    fp32 PSUM accumulation), then attention for query chunk qc=pc while the
    next chunk's projections fill the PE queue.
  - attention processes head pairs: the two K=64 score matmuls are row-packed
    into array row-groups (0,0)/(64,0) writing adjacent PSUM banks so one
    [128, 2*w] Exp activation covers both heads; V carries an extra
    ones-column so PV accumulates the softmax denominator in row 64.
  - 1/denominator via single-pass DVE reciprocal_approx_fast, broadcast over
    the 64 dh partitions with a [1,64]x[1,w] PE outer product.
"""
import os, sys, types

sys.path.insert(0, "/opt/trn_rl_repo")
import numpy as np

import concourse.bass as bass
import concourse.bacc as bacc
import concourse.tile as tile
from concourse import mybir
from concourse.bass_utils import run_bass_kernel_spmd

B, S, D, H, DH = 2, 2048, 1024, 16, 64
N_CORES = 8
HPC = 4            # heads per core
QC = 512           # query chunk width
NQC = S // QC      # 4
KB = 128           # key block
NKB = S // KB      # 16
NDMC = D // 128    # 8 d_model chunks
F16 = mybir.dt.float16
F32 = mybir.dt.float32
F32R = mybir.dt.float32r
GROUPS = [[0, 1, 2, 3], [4, 5, 6, 7]]


def _install_ntff_hook():
    """Register the axon NTFF profiling hook missing from this image's antenv."""
    if "antenv.axon_hooks" in sys.modules:
        return
    try:
        from trn_agent_boot.trn_boot import _ntff_profile_via_ctypes

        hook = _ntff_profile_via_ctypes("/opt/axon/libaxon_pjrt.so")
        if hook is None:
            return
        import antenv  # noqa: F401

        mod = types.ModuleType("antenv.axon_hooks")
        mod.get_axon_ntff_profile_hook = lambda: hook
        sys.modules["antenv.axon_hooks"] = mod
    except Exception:
        pass


def build():
    nc = bacc.Bacc("TRN2", target_bir_lowering=False, debug=False, num_devices=N_CORES)
    # x^T pre-blocked [dmc, 128, pos]; chunk-0 split out so the first
    # projections are gated on 0.5MB instead of the full 4MB per input
    xqT0 = nc.dram_tensor("xqT0", [128, NDMC * QC], F16, kind="ExternalInput")
    xkT0 = nc.dram_tensor("xkT0", [128, NDMC * QC], F16, kind="ExternalInput")
    xvT0 = nc.dram_tensor("xvT0", [128, NDMC * QC], F16, kind="ExternalInput")
    xqT1 = nc.dram_tensor("xqT1", [NDMC, 128, S - QC], F16, kind="ExternalInput")
    xkT1 = nc.dram_tensor("xkT1", [NDMC, 128, S - QC], F16, kind="ExternalInput")
    xvT1 = nc.dram_tensor("xvT1", [NDMC, 128, S - QC], F16, kind="ExternalInput")
    # weights pre-blocked on the host to [128, ...] partition-major so each
    # partition line is one contiguous 4KB DMA descriptor
    wq = nc.dram_tensor("wq", [128, NDMC * HPC * DH], F16, kind="ExternalInput")
    wk = nc.dram_tensor("wk", [128, NDMC * HPC * DH], F16, kind="ExternalInput")
    wv = nc.dram_tensor("wv", [128, NDMC * HPC * DH], F16, kind="ExternalInput")
    wo = nc.dram_tensor("wo", [128, 2 * D], F16, kind="ExternalInput")
    bq = nc.dram_tensor("bq", [HPC, DH], F32, kind="ExternalInput")
    bk = nc.dram_tensor("bk", [HPC, DH], F32, kind="ExternalInput")
    bv = nc.dram_tensor("bv", [HPC, DH], F32, kind="ExternalInput")
    bo = nc.dram_tensor("bo", [D], F32, kind="ExternalInput")
    out = nc.dram_tensor("out", [NQC, KB, D], F32, kind="ExternalOutput")

    tri_np = np.triu(np.ones((128, 128), np.float16))
    tri2_dram = nc.inline_tensor(
        np.ascontiguousarray(np.stack([tri_np, tri_np], axis=1)), name="tri2_c"
    )
    ones_dram = nc.inline_tensor(np.ones((1, DH), np.float32), name="ones_c")

    with tile.TileContext(nc) as tc:
        with (
            tc.tile_pool(name="consts", bufs=1) as consts,
            tc.tile_pool(name="persist", bufs=1) as persist,

            tc.tile_pool(name="work", bufs=2) as work,
            tc.tile_pool(name="pt", bufs=3) as ptp,
            tc.tile_pool(name="zt", bufs=4) as ztp,
            tc.tile_pool(name="zf", bufs=2) as zfp,
            tc.tile_pool(name="ob", bufs=2) as obp,
            tc.tile_pool(name="rec", bufs=2) as recp,
            tc.tile_pool(name="ps_st", bufs=2, space="PSUM") as ps_st,
            tc.tile_pool(name="ps_z", bufs=1, space="PSUM") as ps_z,
            tc.tile_pool(name="ps_mm", bufs=2, space="PSUM") as ps_mm,
            tc.tile_pool(name="dram", bufs=1, space="DRAM") as dram,
        ):
            # ---- constants / weights / biases -------------------------------
            tri2 = consts.tile([128, 2, 128], F16, tag="tri2")
            nc.sync.dma_start(out=tri2, in_=tri2_dram.ap())

            # spread weight loads over the three DMA trigger queues so the
            # first chunk's x tiles are not stuck behind them
            wq_sb = consts.tile([128, NDMC, HPC * DH], F16, tag="wq")
            wk_sb = consts.tile([128, NDMC, HPC * DH], F16, tag="wk")
            wv_sb = consts.tile([128, NDMC, HPC * DH], F16, tag="wv")
            nc.sync.dma_start(
                out=wk_sb, in_=wk.ap().rearrange("p (c e) -> p c e", c=NDMC)
            )
            nc.scalar.dma_start(
                out=wv_sb, in_=wv.ap().rearrange("p (c e) -> p c e", c=NDMC)
            )
            nc.gpsimd.dma_start(
                out=wq_sb, in_=wq.ap().rearrange("p (c e) -> p c e", c=NDMC)
            )
            wo_sb = consts.tile([128, HPC * DH // 128, D], F16, tag="wo")


            # ---- persistent state -------------------------------------------
            qT = persist.tile([128, 2, S], F16, tag="qT")  # [2 heads stacked, hp, pos]
            kT = persist.tile([128, 2, S], F16, tag="kT")
            v_aug = persist.tile([128, NKB, HPC, DH + 1], F16, tag="vaug")
            nc.vector.memset(v_aug[:, :, :, DH : DH + 1], 1.0)

            # HAM warmup: the PE clock-gate defaults to half rate and takes
            # ~3.4us of sustained activity to open. Run zero-data matmuls off
            # a memset tile (no DMA dependency) during the initial input DMA
            # wait so the first real projections execute at full clock.
            wu = work.tile([128, QC], F16, tag="wu")
            nc.vector.memset(wu, 0.0)
            wu_ps = ps_mm.tile([128, QC], F32, tag="mm")
            for _ in range(10):
                nc.tensor.matmul(wu_ps, wu[:, 0:128], wu, start=True, stop=True)

            # full-resident transposed inputs; chunk-0 is one fully-contiguous
            # DMA (8KB lines, lands in ~2us) so the first projections start
            # early; the rest stays per-dmc slabs (3KB lines)
            xk0_sb = persist.tile([128, NDMC, QC], F16, tag="xk0")
            xv0_sb = persist.tile([128, NDMC, QC], F16, tag="xv0")
            xq0_sb = persist.tile([128, NDMC, QC], F16, tag="xq0")
            nc.sync.dma_start(out=xk0_sb, in_=xkT0.ap().rearrange("p (c s) -> p c s", c=NDMC))
            nc.scalar.dma_start(out=xv0_sb, in_=xvT0.ap().rearrange("p (c s) -> p c s", c=NDMC))
            nc.gpsimd.dma_start(out=xq0_sb, in_=xqT0.ap().rearrange("p (c s) -> p c s", c=NDMC))
            xk_sb = persist.tile([128, NDMC, S - QC], F16, tag="xk")
            xv_sb = persist.tile([128, NDMC, S - QC], F16, tag="xv")
            xq_sb = persist.tile([128, NDMC, S - QC], F16, tag="xq")
            for c in range(NDMC):
                nc.sync.dma_start(out=xk_sb[:, c], in_=xkT1.ap()[c])
                nc.scalar.dma_start(out=xv_sb[:, c], in_=xvT1.ap()[c])
                nc.gpsimd.dma_start(out=xq_sb[:, c], in_=xqT1.ap()[c])
            # wo (this core's 4 heads only) is not needed until the first outproj
            nc.scalar.dma_start(
                out=wo_sb, in_=wo.ap().rearrange("p (c d) -> p c d", c=2)
            )
            # biases, after the x slabs they would otherwise delay
            bq_sb = consts.tile([128, 2], F32, tag="bq")
            bk_sb = consts.tile([128, 2], F32, tag="bk")
            for hp in range(2):
                nc.gpsimd.dma_start(
                    out=bq_sb[:, hp : hp + 1],
                    in_=bass.AP(tensor=bq.ap().tensor, offset=128 * hp, ap=[[1, 128], [1, 1]]),
                )
                nc.gpsimd.dma_start(
                    out=bk_sb[:, hp : hp + 1],
                    in_=bass.AP(tensor=bk.ap().tensor, offset=128 * hp, ap=[[1, 128], [1, 1]]),
                )
            bv_sb = consts.tile([128, HPC, DH], F32, tag="bv")
            nc.gpsimd.dma_start(
                out=bv_sb,
                in_=bass.AP(tensor=bv.ap().tensor, offset=0, ap=[[0, 128], [64, HPC], [1, DH]]),
            )
            bo_sb = consts.tile([128, D], F32, tag="bo")
            nc.gpsimd.dma_start(
                out=bo_sb,
                in_=bass.AP(tensor=bo.ap().tensor, offset=0, ap=[[0, 128], [1, D]]),
            )
            ones32 = consts.tile([1, DH], F32, tag="ones32")
            nc.sync.dma_start(out=ones32, in_=ones_dram.ap())
            ones_r = consts.tile([1, DH], F32R, tag="ones")
            nc.vector.tensor_copy(ones_r, ones32)

            # ReduceScatter buffers, split along d_model so early segments'
            # collectives launch while later segments are still computed; the
            # final chunk uses finer segments to shrink the serial tail
            def segs_of(qc):
                return [(0, QC), (QC, QC)]

            rs_bufs = {}
            for qc in range(NQC):
                for si, (lo, wseg) in enumerate(segs_of(qc)):
                    rs_bufs[(qc, si)] = (
                        dram.tile([4, KB, wseg], F16, tag=f"rsi_{qc}_{si}", name=f"rsi_{qc}_{si}"),
                        dram.tile([KB, wseg], F16, tag=f"rso_{qc}_{si}", name=f"rso_{qc}_{si}"),
                    )
            zT_tiles = {}
            deferred = []

            def project(pc):
                sl = slice(QC * pc, QC * (pc + 1))
                s1 = slice(QC * (pc - 1), QC * pc)
                xk_c = (lambda d: xk0_sb[:, d]) if pc == 0 else (lambda d: xk_sb[:, d, s1])
                xq_c = (lambda d: xq0_sb[:, d]) if pc == 0 else (lambda d: xq_sb[:, d, s1])
                for hp in range(2):
                    pj = ps_mm.tile([128, QC], F32, tag="mm")
                    for dmc in range(NDMC):
                        nc.tensor.matmul(
                            pj,
                            wk_sb[:, dmc, 128 * hp : 128 * (hp + 1)],
                            xk_c(dmc),
                            start=(dmc == 0),
                            stop=(dmc == NDMC - 1),
                        )
                    nc.vector.tensor_scalar_add(kT[:, hp, sl], pj, bk_sb[:, hp : hp + 1])
                for pb in range(4):
                    pv_full = ps_mm.tile([128, QC], F32, tag="mm")
                    pv = pv_full[:, 0 : HPC * DH]
                    for dmc in range(NDMC):
                        nc.tensor.matmul(
                            pv,
                            xv0_sb[:, dmc, 128 * pb : 128 * (pb + 1)]
                            if pc == 0
                            else xv_sb[:, dmc, QC * (pc - 1) + 128 * pb : QC * (pc - 1) + 128 * (pb + 1)],
                            wv_sb[:, dmc],
                            start=(dmc == 0),
                            stop=(dmc == NDMC - 1),
                        )
                    nc.vector.tensor_add(
                        v_aug[:, 4 * pc + pb, :, 0:DH],
                        pv.rearrange("p (h e) -> p h e", h=HPC),
                        bv_sb,
                    )
                for hp in range(2):
                    pj = ps_mm.tile([128, QC], F32, tag="mm")
                    for dmc in range(NDMC):
                        nc.tensor.matmul(
                            pj,
                            wq_sb[:, dmc, 128 * hp : 128 * (hp + 1)],
                            xq_c(dmc),
                            start=(dmc == 0),
                            stop=(dmc == NDMC - 1),
                        )
                    nc.vector.tensor_scalar_add(qT[:, hp, sl], pj, bq_sb[:, hp : hp + 1])

            def attention(qc):
                nkb = 4 * qc + 4
                for hp in range(2):
                    zpsA = ps_z.tile([DH + 1, QC], F32, tag="zpsA")
                    zpsB = ps_z.tile([DH + 1, QC], F32, tag="zpsB")
                    # software-pipelined: scores/exp run one key-block ahead of
                    # the PV matmuls so the in-order PE queue never waits on
                    # the exp+mask chain of the block it is about to consume
                    def emit_pv(kb, pt2):
                        m = kb - 4 * qc
                        off = 0 if m < 0 else KB * m
                        w = QC - off
                        nc.tensor.matmul(
                            zpsA[:, off:QC], v_aug[:, kb, 2 * hp], pt2[:, 0, 0:w],
                            start=(kb == 0), stop=(kb == nkb - 1),
                        )
                        nc.tensor.matmul(
                            zpsB[:, off:QC], v_aug[:, kb, 2 * hp + 1], pt2[:, 1, 0:w],
                            start=(kb == 0), stop=(kb == nkb - 1),
                        )

                    prev = None
                    for kb in range(nkb):
                        m = kb - 4 * qc
                        off = 0 if m < 0 else KB * m
                        w = QC - off
                        ksl = slice(KB * kb, KB * (kb + 1))
                        qsl = slice(QC * qc + off, QC * (qc + 1))
                        st2 = ps_st.tile([128, 2, QC], F32, tag="st2")
                        nc.tensor.matmul(
                            st2[:, 0, 0:w], kT[0:64, hp, ksl], qT[0:64, hp, qsl],
                            start=True, stop=True,
                        )
                        nc.tensor.matmul(
                            st2[:, 1, 0:w], kT[64:128, hp, ksl], qT[64:128, hp, qsl],
                            start=True, stop=True,
                        )
                        pt2 = ptp.tile([128, 2, QC], F16, tag="pt2")
                        nc.scalar.activation(
                            pt2[:, :, 0:w],
                            st2[:, :, 0:w],
                            mybir.ActivationFunctionType.Exp,
                            scale=0.125,
                        )
                        if m >= 0:
                            nc.vector.tensor_mul(pt2[:, :, 0:KB], pt2[:, :, 0:KB], tri2)
                        if prev is not None:
                            emit_pv(*prev)
                        prev = (kb, pt2)
                    emit_pv(*prev)
                    # drain zps out of PSUM quickly (unnormalized z + denoms);
                    # the broadcast matmul + multiply are deferred past the
                    # next pair's matmuls so the PE queue never stalls on them
                    zu = work.tile([128, QC], F16, tag="zu")
                    rec_rs = []
                    for sub, zps in ((0, zpsA), (1, zpsB)):
                        nc.vector.tensor_copy(zu[64 * sub : 64 * (sub + 1), :], zps[0:DH, :])
                        den = recp.tile([1, QC], F32, tag="den")
                        nc.vector.tensor_copy(den, zps[DH : DH + 1, :])
                        rec = recp.tile([1, QC], F32, tag="rec")
                        nc.vector.reciprocal_approx_fast(out=rec, in_=den)
                        rec_r = recp.tile([1, QC], F32R, tag="recr")
                        with nc.allow_low_precision(reason="f32r holds full fp32 bits"):
                            nc.vector.tensor_copy(rec_r, rec)
                        rec_rs.append(rec_r)
                    deferred.append((qc, hp, zu, rec_rs))

            def normalize(qc):
                while deferred:
                    _, hp, zu, rec_rs = deferred.pop(0)
                    zT = ztp.tile([128, QC], F16, tag="zT")
                    zT_tiles[(qc, hp)] = zT
                    for sub, tag in ((0, "zpsA"), (1, "zpsB")):
                        bc = ps_z.tile([DH + 1, QC], F32, tag=tag)
                        nc.tensor.matmul(bc[0:DH, :], ones_r, rec_rs[sub], start=True, stop=True)
                        nc.vector.tensor_mul(
                            zT[64 * sub : 64 * (sub + 1), :],
                            zu[64 * sub : 64 * (sub + 1), :],
                            bc[0:DH, :],
                        )

            def outproj(qc):
                zts = (zT_tiles[(qc, 0)], zT_tiles[(qc, 1)])
                dma_eng = (nc.sync, nc.scalar)
                for si, (lo, wseg) in enumerate(segs_of(qc)):
                    rs_i, _ = rs_bufs[(qc, si)]
                    for qb in range(4):
                        ob16 = obp.tile([128, QC], F16, tag="ob16")
                        po = ps_mm.tile([128, QC], F32, tag="mm")
                        for cc in range(2):
                            nc.tensor.matmul(
                                po[:, 0:wseg],
                                zts[cc][:, KB * qb : KB * (qb + 1)],
                                wo_sb[:, cc, lo : lo + wseg],
                                start=(cc == 0),
                                stop=(cc == 1),
                            )
                        nc.scalar.copy(ob16[:, 0:wseg], po[:, 0:wseg])
                        dma_eng[si % 2].dma_start(out=rs_i[qb], in_=ob16[:, 0:wseg])
                    nc.gpsimd.collective_compute(
                        "ReduceScatter",
                        mybir.AluOpType.add,
                        replica_groups=GROUPS,
                        ins=[rs_i.opt()],
                        outs=[rs_bufs[(qc, si)][1].opt()],
                    )

            def finish(qc):
                obf = obp.tile([128, D], F32, tag="obf")
                for si, (lo, wseg) in enumerate(segs_of(qc)):
                    rso = zfp.tile([128, QC], F16, tag="rso")
                    nc.gpsimd.dma_start(out=rso[:, 0:wseg], in_=rs_bufs[(qc, si)][1])
                    nc.vector.tensor_add(
                        obf[:, lo : lo + wseg], rso[:, 0:wseg], bo_sb[:, lo : lo + wseg]
                    )
                    nc.sync.dma_start(
                        out=out.ap()[qc][:, lo : lo + wseg],
                        in_=obf[:, lo : lo + wseg],
                    )

            # next chunk's projections directly follow each attention chunk in
            # the PE queue so the tensor engine never idles long enough for
            # the HAM clock gate to re-throttle it
            project(0)
            attention(0)
            project(1)
            normalize(0)
            outproj(0)
            attention(1)
            project(2)
            normalize(1)
            outproj(1)
            finish(0)
            attention(2)
            project(3)
            normalize(2)
            outproj(2)
            finish(1)
            attention(3)
            normalize(3)
            outproj(3)
            finish(2)
            finish(3)

    nc.finalize()
    return nc


_CACHE = {}


def kernel(**inputs):
    _install_ntff_hook()
    nc = _CACHE.get("nc")
    if nc is None:
        nc = build()
        _CACHE["nc"] = nc

    f16 = np.float16
    xs = {k: np.asarray(inputs[k], np.float32) for k in ("query_input", "key_input", "value_input")}
    W = {k: np.asarray(inputs[k], np.float32) for k in ("W_Q", "W_K", "W_V", "W_O")}
    b = {k: np.asarray(inputs[k], np.float32) for k in ("b_Q", "b_K", "b_V", "b_O")}
    # pre-transpose inputs to [d_model, seq] fp16 so device DMAs are contiguous
    def xsplit(v, g):
        xt = v[g].astype(f16).T.reshape(NDMC, 128, S)
        return (
            np.ascontiguousarray(
                xt[:, :, 0:QC].transpose(1, 0, 2).reshape(128, NDMC * QC)
            ),
            np.ascontiguousarray(xt[:, :, QC:S]),
        )

    xT16 = {k: [xsplit(v, g) for g in range(B)] for k, v in xs.items()}

    def wslice(wt, h0):
        # [H, D, DH] -> this core's heads, d_model-major, then partition-major
        # blocked: row p holds [dmc, head, e] concatenated (one 4KB DMA line)
        a = wt[h0 : h0 + HPC].transpose(1, 0, 2).reshape(D, HPC * DH)
        a = a.reshape(NDMC, 128, HPC * DH).transpose(1, 0, 2).reshape(128, NDMC * HPC * DH)
        return np.ascontiguousarray(a.astype(f16))

    in_maps = []
    for i in range(N_CORES):
        g, h0 = i // 4, 4 * (i % 4)
        in_maps.append(
            {
                "xqT0": xT16["query_input"][g][0],
                "xqT1": xT16["query_input"][g][1],
                "xkT0": xT16["key_input"][g][0],
                "xkT1": xT16["key_input"][g][1],
                "xvT0": xT16["value_input"][g][0],
                "xvT1": xT16["value_input"][g][1],
                "wq": wslice(W["W_Q"], h0),
                "wk": wslice(W["W_K"], h0),
                "wv": wslice(W["W_V"], h0),
                "wo": np.ascontiguousarray(
                    W["W_O"][h0 : h0 + HPC]
                    .reshape(2, 128, D)
                    .transpose(1, 0, 2)
                    .reshape(128, 2 * D)
                    .astype(f16)
                ),
                "bq": np.ascontiguousarray(b["b_Q"][h0 : h0 + HPC]),
                "bk": np.ascontiguousarray(b["b_K"][h0 : h0 + HPC]),
                "bv": np.ascontiguousarray(b["b_V"][h0 : h0 + HPC]),
                "bo": np.ascontiguousarray(b["b_O"]),
            }
        )

    res = run_bass_kernel_spmd(nc, in_maps, core_ids=list(range(N_CORES)))
    if os.environ.get("KERNEL_PRINT_EXEC"):
        print(f"HW exec time: {res.exec_time_ns} ns")
    full = np.empty((B, S, D), np.float32)
    for i in range(N_CORES):
        g, j = i // 4, i % 4
        o = np.asarray(res.results[i]["out"], np.float32)
        for qc in range(NQC):
            full[g, QC * qc + KB * j : QC * qc + KB * (j + 1)] = o[qc]
    return full
